# revision 1
# baseline (speedup 1.0000x reference)
"""Log2Quantizer Trainium2 kernel (raw Bass, no Tile).

Math: the reference's sort/std/rank machinery is dead code (bit_token is
unconditionally overwritten with n_bits), so the computation reduces to:
    delta[b,t] = max over (h,c) of x[b,h,t,c]
    out = delta * 2^(round(log2(max(x/delta, 1e-8))))
i.e. snap x/delta to the nearest power of two in log space, rescale by delta.

Division-route bit-trick (no transcendentals), exact on the fp32-internal DVE:
    q  = (x * (1/delta)) * (1/sqrt2)         (reciprocal is IEEE 1/x on trn2)
    p2 = bitcast_f32(bits(q) & 0x7F800000)   # 2^floor(log2 q) = 2^(k-1)
    out = p2 * (2*delta)                     # fp32 mult by 2^k, exact
round(log2(x/delta)) = floor(log2(x/(delta*sqrt2))) + 1, so flooring q to its
exponent implements the rounding; x==0 gives q=0 -> p2=+0.0 -> out=0 (the
reference's 1e-8 ratio clamp yields delta*2^-27 ~ 7e-9 there; abs err 7e-9).

Sharding: data-parallel over batch dim b (8 rows -> 8 cores), no comms.
Layout: t split into TC=512-token chunks; partition dim = t-block of 4 so each
partition line is one contiguous 1KB run per h in DRAM (fast DMA). Compute
sub-steps each chunk into 4 x 128-token slices where per-token scalars are
per-partition [128,1] APs -> tensor_scalar runs in the DVE's 2x port mode.

Engines (all compute on DVE: concurrent GpSimd work stalls DVE 2x-port ops
via SBUF port contention, so offloading to gp was a net loss):
  Sync (SP HWDGE ring):    loads
  Scalar (ACT HWDGE ring): stores  (separate FIFO so loads never queue
                           behind stores; ACT is otherwise idle)
  DVE:  fused (h,c) max-reduce, reciprocal, d2, M1 x4 (2x), AND (2x), M2 x4 (2x)
Sems (one update per instruction; DVE ops overlap in the pipe unless a
dependent op waits on the producer's counting-sem increment - verified
corrupting on HW without the fences):
  dve_sem: +1 per DVE op except the last M2 slice (11/chunk)
  v_sem:   +1 by the last M2 slice; stores wait it; loads wait it NBUF back
  load_sem/store_sem[NBUF]: per-slot DMA completion (16/DMA)
"""

from contextlib import ExitStack

import numpy as np

import concourse.bass as bass
import concourse.mybir as mybir
from concourse.bass_utils import run_bass_kernel_spmd

B, H, T, C = 8, 12, 4096, 64
N_CORES = 8
P = 128          # SBUF partitions
TC = 512         # tokens per chunk (pipeline granularity)
NBUF = 4         # xt/wt buffer depth

ISQRT2 = 0.7071067811865476
EXP_MASK = 0x7F800000
DVE_INCS = 11

_nc_cache = {}


def _build_nc():
    if "nc" in _nc_cache:
        return _nc_cache["nc"]
    f32 = mybir.dt.float32
    i32 = mybir.dt.int32
    OP = mybir.AluOpType

    nc = bass.Bass()
    x_in = nc.declare_dram_parameter("x", [H, T, C], f32, isOutput=False)
    y_out = nc.declare_dram_parameter("y", [H, T, C], f32, isOutput=True)

    n_chunks = T // TC
    tt = TC // P
    FREE = H * tt * C

    def src_ap(ci):
        return x_in[:, ci * TC : (ci + 1) * TC, :].rearrange(
            "h (p q) c -> p h (q c)", p=P
        )

    def dst_ap(ci):
        return y_out[:, ci * TC : (ci + 1) * TC, :].rearrange(
            "h (p q) c -> p h (q c)", p=P
        )

    with ExitStack() as ctx:
        xt = [
            ctx.enter_context(nc.sbuf_tensor(f"xt{j}", [P, FREE], f32))
            for j in range(NBUF)
        ]
        wt = [
            ctx.enter_context(nc.sbuf_tensor(f"wt{j}", [P, FREE], f32))
            for j in range(NBUF)
        ]
        delta = ctx.enter_context(nc.sbuf_tensor("delta", [P, tt], f32))
        inv = ctx.enter_context(nc.sbuf_tensor("inv", [P, tt], f32))
        # d2 is read by the v_sem-signaling M2 slice, whose completion the
        # DVE's own counting sem never proves -> per-slot copies, recycled
        # only after the store chain confirms the whole chunk finished
        d2 = [
            ctx.enter_context(nc.sbuf_tensor(f"d2_{j}", [P, tt], f32))
            for j in range(NBUF)
        ]

        load_sem = [
            ctx.enter_context(nc.semaphore(f"load_sem{j}")) for j in range(NBUF)
        ]
        store_sem = [
            ctx.enter_context(nc.semaphore(f"store_sem{j}")) for j in range(NBUF)
        ]
        v_sem = ctx.enter_context(nc.semaphore("v_sem"))
        dve_sem = ctx.enter_context(nc.semaphore("dve_sem"))

        block = ctx.enter_context(nc.Block())

        @block.sync
        def _(sync):
            # loads only; SP HWDGE ring
            for ci in range(n_chunks):
                if ci >= NBUF:
                    # xt slot's last readers are chunk ci-NBUF's M2 slices:
                    # the last slice incs v_sem, the others dve_sem - wait both
                    sync.wait_ge(v_sem, ci - NBUF + 1)
                    sync.wait_ge(dve_sem, DVE_INCS * (ci - NBUF + 1))
                sync.dma_start(out=xt[ci % NBUF][:], in_=src_ap(ci)).then_inc(
                    load_sem[ci % NBUF], 16
                )

        @block.scalar
        def _(scalar):
            # stores only; ACT HWDGE ring (independent FIFO from loads)
            for ci in range(n_chunks):
                # all four M2 slices must have written wt: last slice incs
                # v_sem, the other three are covered by the chunk's full
                # dve_sem count
                scalar.wait_ge(v_sem, ci + 1)
                scalar.wait_ge(dve_sem, DVE_INCS * (ci + 1))
                scalar.dma_start(out=dst_ap(ci), in_=wt[ci % NBUF][:]).then_inc(
                    store_sem[ci % NBUF], 16
                )

        @block.vector
        def _(vector):
            for ci in range(n_chunks):
                j = ci % NBUF
                xt4 = xt[j][:].rearrange("p (h q c) -> p h q c", h=H, c=C)
                wt4 = wt[j][:].rearrange("p (h q c) -> p h q c", h=H, c=C)

                if ci >= NBUF:
                    vector.wait_ge(store_sem[j], 16 * (ci // NBUF))  # wt free
                vector.wait_ge(load_sem[j], 16 * (ci // NBUF + 1))   # xt loaded
                if ci >= 1:
                    # delta WAR: prior chunk's recip/d2 (incs b-9, b-8) must
                    # have read delta before this chunk's reduce rewrites it
                    vector.wait_ge(dve_sem, DVE_INCS * ci - 8)

                b = DVE_INCS * ci
                # delta = max over (h, c): one XY reduce on the [p, q, h, c]
                # transposed view (h, c are the two trailing axes)
                vector.reduce_max(
                    out=delta[:],
                    in_=xt4.transpose([0, 2, 1, 3]),
                    axis=mybir.AxisListType.XY,
                ).then_inc(dve_sem, 1)
                # per-token scalars: inv = 1/delta, d2 = 2*delta
                vector.wait_ge(dve_sem, b + 1)
                vector.reciprocal(inv[:], delta[:]).then_inc(dve_sem, 1)
                vector.tensor_scalar_mul(d2[j][:], delta[:], 2.0).then_inc(dve_sem, 1)

                # M1: q = (x * inv) * (1/sqrt2), sub-stepped so the inv slice
                # is a [128,1] per-partition scalar -> DVE 2x port mode;
                # 1/sqrt2 rides the second scalar-op slot
                vector.wait_ge(dve_sem, b + 2)                   # recip done
                for s in range(tt):
                    vector.tensor_scalar(
                        out=wt4[:, :, s, :],
                        in0=xt4[:, :, s, :],
                        scalar1=inv[:, s : s + 1],
                        scalar2=ISQRT2,
                        op0=OP.mult,
                        op1=OP.mult,
                    ).then_inc(dve_sem, 1)
                # AND: p2 = bits(q) & 0x7F800000  (wt -> xt, xt dead after M1)
                vector.wait_ge(dve_sem, b + 3 + tt)              # all M1 done
                vector.tensor_scalar(
                    out=xt[j][:].bitcast(i32),
                    in0=wt[j][:].bitcast(i32),
                    scalar1=EXP_MASK,
                    scalar2=None,
                    op0=OP.bitwise_and,
                ).then_inc(dve_sem, 1)
                # M2: out = p2 * 2*delta  (xt -> wt), sub-stepped like M1;
                # the last slice signals v_sem for the store
                vector.wait_ge(dve_sem, b + 4 + tt)              # AND done
                for s in range(tt):
                    inst = vector.tensor_scalar_mul(
                        wt4[:, :, s, :], xt4[:, :, s, :], d2[j][:, s : s + 1]
                    )
                    inst.then_inc(v_sem if s == tt - 1 else dve_sem, 1)

    _nc_cache["nc"] = nc
    return nc


def kernel(x: np.ndarray) -> np.ndarray:
    assert x.shape == (B, H, T, C) and x.dtype == np.float32
    nc = _build_nc()
    in_maps = [{"x": np.ascontiguousarray(x[i])} for i in range(N_CORES)]
    res = run_bass_kernel_spmd(nc, in_maps, list(range(N_CORES)))
    out = np.stack([res.results[i]["y"] for i in range(N_CORES)], axis=0)
    return out



# revision 3
# speedup vs baseline: 1.0925x; 1.0925x over previous
"""Log2Quantizer Trainium2 kernel (raw Bass, no Tile).

Math: the reference's sort/std/rank machinery is dead code (bit_token is
unconditionally overwritten with n_bits), so the computation reduces to:
    delta[b,t] = max over (h,c) of x[b,h,t,c]
    out = delta * 2^(round(log2(max(x/delta, 1e-8))))
i.e. snap x/delta to the nearest power of two in log space, rescale by delta.

Division-route bit-trick (no transcendentals), exact on the fp32-internal DVE:
    q  = x * (isqrt2/delta)                  (reciprocal is IEEE 1/x on trn2)
    p2 = bitcast_f32(bits(q) & 0x7F800000)   # 2^floor(log2 q) = 2^(k-1)
    out = p2 * (2*delta)                     # fp32 mult by 2^k, exact
round(log2(x/delta)) = floor(log2(x/(delta*sqrt2))) + 1, so flooring q to its
exponent implements the rounding; x==0 gives q=0 -> p2=+0.0 -> out=0 (the
reference's 1e-8 ratio clamp yields delta*2^-27 ~ 7e-9 there; abs err 7e-9).

Sharding: data-parallel over batch dim b (8 rows -> 8 cores), no comms.
Layout: t split into TC=512-token chunks; partition dim = t-block of 4 so each
partition line is one contiguous 1KB run per h in DRAM (fast DMA).

Engine split (vs the previous all-DVE version): the q = x*invs multiply (M1)
runs on the otherwise-idle ACT engine (activation Copy with per-partition
scale AP), cutting DVE busy from ~77us to ~56us so the 8 chunks pipeline at
the DMA rate (~73us roofline: 12.6MB in + 12.6MB out at ~345 GB/s/core).

Buffers are 8-deep (one slot per chunk, 2x12KB x 8 = 192KB/partition), so no
slot is ever recycled: all 8 loads queue on the SP HWDGE ring at t=0 and the
DMA engines never starve on the load side. Stores go out on the ACT ring.

Per chunk ci (python-counted semaphore targets, fences per the DVE
pipe-overlap hazard: a dependent op must wait on its producer's inc):
  Sync:  load(ci) -> xt[ci]                                +16 load_sem
  DVE:   [wait load(ci); wait d2(ci-1) (delta WAR)]
         reduce_max (h,c) -> delta        [fence]
         inv[ci] = 1/delta; inv[ci] *= isqrt2; d2[ci] = 2*delta
         [wait act M1(ci-1)] AND: xt[ci-1] = bits(wt[ci-1]) & EXPMASK
         [fence] M2 x4: wt[ci-1] = xt[ci-1] * d2[ci-1]  (2x port mode)
  ACT:   [wait invs(ci)] M1 x4: wt[ci] = Copy(xt[ci] * inv[ci])
         [wait M2(ci-1)] store(ci-1) <- wt[ci-1]          +16 store_sem
"""

from contextlib import ExitStack

import numpy as np

import concourse.bass as bass
import concourse.mybir as mybir
from concourse.bass_utils import run_bass_kernel_spmd

B, H, T, C = 8, 12, 4096, 64
N_CORES = 8
P = 128          # SBUF partitions
TC = 512         # tokens per chunk
NCHUNK = T // TC # 8 chunks, one SBUF slot each (no recycling)

ISQRT2 = 0.7071067811865476
EXP_MASK = 0x7F800000

_nc_cache = {}


def _build_nc():
    if "nc" in _nc_cache:
        return _nc_cache["nc"]
    f32 = mybir.dt.float32
    i32 = mybir.dt.int32
    OP = mybir.AluOpType
    Copy = mybir.ActivationFunctionType.Copy

    nc = bass.Bass()
    x_in = nc.declare_dram_parameter("x", [H, T, C], f32, isOutput=False)
    y_out = nc.declare_dram_parameter("y", [H, T, C], f32, isOutput=True)

    tt = TC // P          # tokens per partition per chunk (4)
    FREE = H * tt * C     # 3072 floats = 12KB per partition per chunk

    def src_ap(ci):
        return x_in[:, ci * TC : (ci + 1) * TC, :].rearrange(
            "h (p q) c -> p h (q c)", p=P
        )

    def dst_ap(ci):
        return y_out[:, ci * TC : (ci + 1) * TC, :].rearrange(
            "h (p q) c -> p h (q c)", p=P
        )

    with ExitStack() as ctx:
        xt = [
            ctx.enter_context(nc.sbuf_tensor(f"xt{j}", [P, FREE], f32))
            for j in range(NCHUNK)
        ]
        wt = [
            ctx.enter_context(nc.sbuf_tensor(f"wt{j}", [P, FREE], f32))
            for j in range(NCHUNK)
        ]
        delta = ctx.enter_context(nc.sbuf_tensor("delta", [P, tt], f32))
        inv = [
            ctx.enter_context(nc.sbuf_tensor(f"inv{j}", [P, tt], f32))
            for j in range(NCHUNK)
        ]
        d2 = [
            ctx.enter_context(nc.sbuf_tensor(f"d2_{j}", [P, tt], f32))
            for j in range(NCHUNK)
        ]

        load_sem = ctx.enter_context(nc.semaphore("load_sem"))
        store_sem = ctx.enter_context(nc.semaphore("store_sem"))
        act_sem = ctx.enter_context(nc.semaphore("act_sem"))
        dve_sem = ctx.enter_context(nc.semaphore("dve_sem"))

        # python-side counters -> absolute wait targets, no formulas
        dve_n = 0
        invs_done = [0] * NCHUNK   # dve_sem count proving inv[ci] (scaled) ready
        d2_done = [0] * NCHUNK     # dve_sem count proving d2[ci] ready
        m2_done = [0] * NCHUNK     # dve_sem count proving M2(ci) (output) done
        m1_done = [4 * (ci + 1) for ci in range(NCHUNK)]  # act_sem after M1(ci)

        block = ctx.enter_context(nc.Block())

        @block.sync
        def _(sync):
            for ci in range(NCHUNK):
                sync.dma_start(out=xt[ci][:], in_=src_ap(ci)).then_inc(
                    load_sem, 16
                )
            sync.wait_ge(store_sem, 16 * NCHUNK)  # final store fence

        @block.vector
        def _(vector):
            def emit_front(ci):
                # reduce + per-token scalars for chunk ci
                nonlocal dve_n
                vector.wait_ge(load_sem, 16 * (ci + 1))
                if ci >= 1:
                    # delta WAR: prior chunk's d2 must have read delta
                    vector.wait_ge(dve_sem, d2_done[ci - 1])
                xt4 = xt[ci][:].rearrange("p (h q c) -> p h q c", h=H, c=C)
                vector.reduce_max(
                    out=delta[:],
                    in_=xt4.transpose([0, 2, 1, 3]),
                    axis=mybir.AxisListType.XY,
                ).then_inc(dve_sem, 1)
                dve_n += 1
                vector.wait_ge(dve_sem, dve_n)  # fence: recip reads delta
                vector.reciprocal(inv[ci][:], delta[:]).then_inc(dve_sem, 1)
                dve_n += 1
                vector.wait_ge(dve_sem, dve_n)  # fence: in-place scale of inv
                vector.tensor_scalar_mul(inv[ci][:], inv[ci][:], ISQRT2).then_inc(
                    dve_sem, 1
                )
                dve_n += 1
                invs_done[ci] = dve_n
                vector.tensor_scalar_mul(d2[ci][:], delta[:], 2.0).then_inc(
                    dve_sem, 1
                )
                dve_n += 1
                d2_done[ci] = dve_n

            def emit_back(ci):
                # AND + M2 for chunk ci (after ACT finished M1(ci))
                nonlocal dve_n
                vector.wait_ge(act_sem, m1_done[ci])
                vector.tensor_scalar(
                    out=xt[ci][:].bitcast(i32),
                    in0=wt[ci][:].bitcast(i32),
                    scalar1=EXP_MASK,
                    scalar2=None,
                    op0=OP.bitwise_and,
                ).then_inc(dve_sem, 1)
                dve_n += 1
                vector.wait_ge(dve_sem, dve_n)  # fence: M2 reads AND output
                xt4 = xt[ci][:].rearrange("p (h q c) -> p h q c", h=H, c=C)
                wt4 = wt[ci][:].rearrange("p (h q c) -> p h q c", h=H, c=C)
                for s in range(tt):
                    vector.tensor_scalar_mul(
                        wt4[:, :, s, :], xt4[:, :, s, :], d2[ci][:, s : s + 1]
                    ).then_inc(dve_sem, 1)
                    dve_n += 1
                m2_done[ci] = dve_n

            emit_front(0)
            for ci in range(1, NCHUNK):
                emit_front(ci)
                emit_back(ci - 1)
            emit_back(NCHUNK - 1)

        @block.scalar
        def _(scalar):
            def emit_m1(ci):
                scalar.wait_ge(dve_sem, invs_done[ci])
                xt4 = xt[ci][:].rearrange("p (h q c) -> p h q c", h=H, c=C)
                wt4 = wt[ci][:].rearrange("p (h q c) -> p h q c", h=H, c=C)
                for s in range(tt):
                    scalar.activation(
                        wt4[:, :, s, :],
                        xt4[:, :, s, :],
                        Copy,
                        bias=0.0,
                        scale=inv[ci][:, s : s + 1],
                    ).then_inc(act_sem, 1)

            def emit_store(ci):
                scalar.wait_ge(dve_sem, m2_done[ci])
                scalar.dma_start(out=dst_ap(ci), in_=wt[ci][:]).then_inc(
                    store_sem, 16
                )

            emit_m1(0)
            for ci in range(1, NCHUNK):
                emit_m1(ci)
                emit_store(ci - 1)
            emit_store(NCHUNK - 1)

    _nc_cache["nc"] = nc
    return nc


def kernel(x: np.ndarray) -> np.ndarray:
    assert x.shape == (B, H, T, C) and x.dtype == np.float32
    nc = _build_nc()
    in_maps = [{"x": np.ascontiguousarray(x[i])} for i in range(N_CORES)]
    res = run_bass_kernel_spmd(nc, in_maps, list(range(N_CORES)))
    out = np.stack([res.results[i]["y"] for i in range(N_CORES)], axis=0)
    return out


# revision 6
# speedup vs baseline: 1.1610x; 1.0627x over previous
"""Log2Quantizer Trainium2 kernel (raw Bass, no Tile).

Math: the reference's sort/std/rank machinery is dead code (bit_token is
unconditionally overwritten with n_bits), so the computation reduces to:
    delta[b,t] = max over (h,c) of x[b,h,t,c]
    out = delta * 2^(round(log2(max(x/delta, 1e-8))))
i.e. snap x/delta to the nearest power of two in log space, rescale by delta.

Division-route bit-trick (no transcendentals), exact on fp32 engines:
    q  = x * (1/(delta*sqrt2))               (reciprocal is IEEE 1/x on trn2)
    p2 = bitcast_f32(bits(q) & 0x7F800000)   # 2^floor(log2 q) = 2^(k-1)
    out = p2 * (2*delta)                     # fp32 mult by 2^k, exact
round(log2(x/delta)) = floor(log2(x/(delta*sqrt2))) + 1, so flooring q to its
exponent implements the rounding; x==0 gives q=0 -> p2=+0.0 -> out~0.

Sharding: data-parallel over batch dim b (8 rows -> 8 cores), no comms.

Perf structure (target: DMA roofline ~70us = 12.6MB in + 12.6MB out at the
~358GB/s HBM-per-core limit; measured DMA busy-rate is ~363GB/s):
  - Tokens split into chunks, TAPERED at both ends (128/256-token chunks
    first/last, 512 in the middle) so the first reduce starts ~5us earlier
    and the last store's dependency chain is short.
  - One SBUF slot per chunk in two big [P, 24KB] arenas (xt: input then
    AND output; wt: q then final output) -> no recycling, all loads queue
    on the SP HWDGE ring at t=0, stores go out on the ACT ring.
  - Compute split so neither engine paces below the DMA rate:
      DVE (0.96GHz): reduce_max (1x), tiny per-token scalars, AND (2x),
                     and HALF the M2 (p2*2delta) slices (2x port mode)
      ACT (1.2GHz):  all M1 (q = x*invs, Copy w/ per-partition scale AP),
                     the other half of M2, and store issue
    ~6.8us/chunk on DVE vs ~5.8us on ACT vs ~8.3us/chunk of DMA.
  - Explicit fences (wait on own counting sem) between dependent DVE ops;
    cross-engine deps via python-counted absolute semaphore targets.
"""

from contextlib import ExitStack

import numpy as np

import concourse.bass as bass
import concourse.mybir as mybir
from concourse.bass_utils import run_bass_kernel_spmd

B, H, T, C = 8, 12, 4096, 64
N_CORES = 8
P = 128

# chunk sizes (tokens); tapered ends, sum = T
TCS = [128, 128, 256, 512, 512, 512, 512, 512, 512, 256, 128, 128]
assert sum(TCS) == T and all(tc % P == 0 for tc in TCS)
NCH = len(TCS)
TTS = [tc // P for tc in TCS]              # tokens per partition per chunk
OFFS = np.cumsum([0] + TTS).tolist()       # per-partition token offsets

SQRT2 = 1.4142135623730951
EXP_MASK = 0x7F800000
TPP = T // P                               # 32 tokens per partition total

_nc_cache = {}


def _build_nc():
    if "nc" in _nc_cache:
        return _nc_cache["nc"]
    f32 = mybir.dt.float32
    i32 = mybir.dt.int32
    OP = mybir.AluOpType
    Copy = mybir.ActivationFunctionType.Copy

    nc = bass.Bass()
    x_in = nc.declare_dram_parameter("x", [H, T, C], f32, isOutput=False)
    y_out = nc.declare_dram_parameter("y", [H, T, C], f32, isOutput=True)

    def dram_ap(t, ci):
        t0 = OFFS[ci] * P
        return t[:, t0 : t0 + TCS[ci], :].rearrange(
            "h (p q) c -> p h (q c)", p=P
        )

    # ACT-side op schedule (python dry-run): per chunk, M1 x tt, then (for
    # the previous chunk) M2act x (tt - tt//2); needed by DVE's AND wait and
    # by the store's self-fence (a dma_start does NOT wait for the issuing
    # engine's in-flight compute, so the store must wait act_sem).
    m1_done = [0] * NCH
    m2act_done = [0] * NCH
    actn = 0
    for ci in range(NCH):
        actn += TTS[ci]
        m1_done[ci] = actn
        if ci >= 1:
            actn += TTS[ci - 1] - TTS[ci - 1] // 2
            m2act_done[ci - 1] = actn
    actn += TTS[NCH - 1] - TTS[NCH - 1] // 2
    m2act_done[NCH - 1] = actn

    with ExitStack() as ctx:
        xt = ctx.enter_context(nc.sbuf_tensor("xt", [P, TPP * H * C], f32))
        wt = ctx.enter_context(nc.sbuf_tensor("wt", [P, TPP * H * C], f32))
        xt_i = xt[:].bitcast(i32)
        wt_i = wt[:].bitcast(i32)
        delta = ctx.enter_context(nc.sbuf_tensor("delta", [P, max(TTS)], f32))
        ds = ctx.enter_context(nc.sbuf_tensor("ds", [P, max(TTS)], f32))
        inv = ctx.enter_context(nc.sbuf_tensor("inv", [P, TPP], f32))
        d2 = ctx.enter_context(nc.sbuf_tensor("d2", [P, TPP], f32))

        load_sem = ctx.enter_context(nc.semaphore("load_sem"))
        store_sem = ctx.enter_context(nc.semaphore("store_sem"))
        act_sem = ctx.enter_context(nc.semaphore("act_sem"))
        dve_sem = ctx.enter_context(nc.semaphore("dve_sem"))

        # python-side counters -> absolute wait targets
        dve_n = 0
        recip_done = [0] * NCH
        and_done = [0] * NCH
        m2dve_done = [0] * NCH

        def csl(ci):
            # chunk slice in the big arenas + 4D views + per-token scalars
            tt = TTS[ci]
            off = OFFS[ci] * H * C
            sz = tt * H * C
            return (
                tt,
                OFFS[ci],
                xt[:, off : off + sz],
                wt[:, off : off + sz],
                xt_i[:, off : off + sz],
                wt_i[:, off : off + sz],
                xt[:, off : off + sz].rearrange("p (h q c) -> p h q c", h=H, c=C),
                wt[:, off : off + sz].rearrange("p (h q c) -> p h q c", h=H, c=C),
            )

        block = ctx.enter_context(nc.Block())

        @block.sync
        def _(sync):
            for ci in range(NCH):
                sync.dma_start(out=csl(ci)[2], in_=dram_ap(x_in, ci)).then_inc(
                    load_sem, 16
                )
            sync.wait_ge(store_sem, 16 * NCH)  # final store fence

        @block.vector
        def _(vector):
            def emit_front(ci):
                nonlocal dve_n
                tt, toff, xs, ws, xsi, wsi, xs4, ws4 = csl(ci)
                vector.wait_ge(load_sem, 16 * (ci + 1))
                if ci >= 1:
                    # WAR on delta/ds: prior chunk's front must be complete
                    vector.wait_ge(dve_sem, recip_done[ci - 1])
                vector.reduce_max(
                    out=delta[:, 0:tt],
                    in_=xs4.transpose([0, 2, 1, 3]),
                    axis=mybir.AxisListType.XY,
                ).then_inc(dve_sem, 1)
                dve_n += 1
                vector.wait_ge(dve_sem, dve_n)  # fence: ds/d2 read delta
                vector.tensor_scalar_mul(
                    ds[:, 0:tt], delta[:, 0:tt], SQRT2
                ).then_inc(dve_sem, 1)
                dve_n += 1
                vector.tensor_scalar_mul(
                    d2[:, toff : toff + tt], delta[:, 0:tt], 2.0
                ).then_inc(dve_sem, 1)
                dve_n += 1
                vector.wait_ge(dve_sem, dve_n)  # fence: recip reads ds
                vector.reciprocal(
                    inv[:, toff : toff + tt], ds[:, 0:tt]
                ).then_inc(dve_sem, 1)
                dve_n += 1
                recip_done[ci] = dve_n

            def emit_back(ci):
                nonlocal dve_n
                tt, toff, xs, ws, xsi, wsi, xs4, ws4 = csl(ci)
                vector.wait_ge(act_sem, m1_done[ci])
                vector.tensor_scalar(
                    out=xsi,
                    in0=wsi,
                    scalar1=EXP_MASK,
                    scalar2=None,
                    op0=OP.bitwise_and,
                ).then_inc(dve_sem, 1)
                dve_n += 1
                and_done[ci] = dve_n
                k = tt // 2
                if k:
                    vector.wait_ge(dve_sem, dve_n)  # fence: M2 reads AND out
                    for s in range(k):
                        vector.tensor_scalar_mul(
                            ws4[:, :, s, :],
                            xs4[:, :, s, :],
                            d2[:, toff + s : toff + s + 1],
                        ).then_inc(dve_sem, 1)
                        dve_n += 1
                m2dve_done[ci] = dve_n

            emit_front(0)
            for ci in range(1, NCH):
                emit_front(ci)
                emit_back(ci - 1)
            emit_back(NCH - 1)

        @block.scalar
        def _(scalar):
            def emit_m1(ci):
                tt, toff, xs, ws, xsi, wsi, xs4, ws4 = csl(ci)
                scalar.wait_ge(dve_sem, recip_done[ci])
                for s in range(tt):
                    scalar.activation(
                        ws4[:, :, s, :],
                        xs4[:, :, s, :],
                        Copy,
                        bias=0.0,
                        scale=inv[:, toff + s : toff + s + 1],
                    ).then_inc(act_sem, 1)

            def emit_m2act_store(ci):
                tt, toff, xs, ws, xsi, wsi, xs4, ws4 = csl(ci)
                k = tt // 2
                scalar.wait_ge(dve_sem, and_done[ci])
                for s in range(k, tt):
                    scalar.activation(
                        ws4[:, :, s, :],
                        xs4[:, :, s, :],
                        Copy,
                        bias=0.0,
                        scale=d2[:, toff + s : toff + s + 1],
                    ).then_inc(act_sem, 1)
                if k:
                    scalar.wait_ge(dve_sem, m2dve_done[ci])
                # self-fence: the M2act slices above must have COMPLETED
                # (not merely issued) before the store DMA reads ws
                scalar.wait_ge(act_sem, m2act_done[ci])
                scalar.dma_start(out=dram_ap(y_out, ci), in_=ws).then_inc(
                    store_sem, 16
                )

            emit_m1(0)
            for ci in range(1, NCH):
                emit_m1(ci)
                emit_m2act_store(ci - 1)
            emit_m2act_store(NCH - 1)

    _nc_cache["nc"] = nc
    return nc


def kernel(x: np.ndarray) -> np.ndarray:
    assert x.shape == (B, H, T, C) and x.dtype == np.float32
    nc = _build_nc()
    in_maps = [{"x": np.ascontiguousarray(x[i])} for i in range(N_CORES)]
    res = run_bass_kernel_spmd(nc, in_maps, list(range(N_CORES)))
    out = np.stack([res.results[i]["y"] for i in range(N_CORES)], axis=0)
    return out
